# revision 1
# baseline (speedup 1.0000x reference)
"""Trainium2 Bass kernel for nn_GCNNDiagGaussianActor.

Key structural insight: the reference GNN runs GCNConv layers over a COMPLETE
graph of 32 nodes per sample with self-loops. Every node therefore has degree
exactly 32 and the symmetric GCN normalization is the constant
norm = rsqrt(32)^2 ~= 1/32 for every edge. The gather + segment_sum message
passing collapses to a per-graph mean over nodes, broadcast back to every
node. The whole network reduces to, per graph g:

    pooled = sum_n obs[g, n, 2:16]                  (node-mean fused into W1)
    h1  = relu(pooled @ (W1 * norm) + b1)
    h2  = relu(h1 @ (W2 * 32 * norm) + b2)
    m   = relu(h2 @ Wm1 + bm1)
    o   = m @ Wm2 + bm2                              -> [4] per graph
    mu  = o[:2];  std = exp(3.5 * tanh(o[2:]) - 1.5)
    out[0, g] = tile(mu, 32); out[1, g] = tile(std, 32)

Sharding: data-parallel over the batch. 1024 graphs / 8 cores = 128 graphs
per core = exactly the 128 SBUF partitions. Weights are replicated. The x32
node replication of the output is folded into the final matmul by replicating
Wm2's columns host-side, so the last GEMM directly produces the [128, 64]
output planes in graph-major layout.

Perf notes (v3):
- 3 input DMAs (obs / packed weights / W1p) — per-DMA engine+completion cost
  is ~600ns + ~2us regardless of size, so batch hard.
- node pooling = one strided tensor_reduce over only the 14 used features.
- pooled [128,16] -> [16,128] transpose via 4 DVE 32x32 block transposes
  (no identity matrix, no gpsimd, no PSUM round-trip).
- relu+bias fused on the vector engine via tensor_scalar.
- no device-side bm2: the mu plane gets bm2 added on the host (exact), the
  log_std plane applies bm2 inside tanh as a per-partition bias using
  host-replicated bias columns and an even/odd column split (out[.., 2n+c]
  shares bias bm2[2+c]).
- dummy tanh right after the DMAs kick off hoists the scalar engine's
  ACT_TABLE_LOAD (~1.3us) off the critical path.
- mu output DMA issues while the std tanh/exp still run.
"""

import numpy as np

NCORES = 8
BS = 1024
BS_LOCAL = BS // NCORES   # 128 graphs per core
NN = 32                   # nodes per graph
FD = 16                   # per-node obs width
OBS_W = NN * FD           # 512
H = 128                   # hidden width
OUT_W = 2 * NN            # 64 = ACT_DIM * NN
WPK = 3 * H + 5           # wpack cols: W2s | Wm1 | Wm2r | b1 b2 bm1 bt0 bt1

_NC_CACHE = {}


def _build_bass():
    import concourse.bacc as bacc
    import concourse.mybir as mybir
    from concourse import tile

    fp32 = mybir.dt.float32
    AF = mybir.ActivationFunctionType
    ALU = mybir.AluOpType

    nc = bacc.Bacc(None, target_bir_lowering=False)
    obs = nc.declare_dram_parameter("obs", [BS_LOCAL, OBS_W], fp32, isOutput=False)
    # packed: cols 0:128 W2s | 128:256 Wm1 | 256:384 Wm2r | 384 b1 | 385 b2 |
    # 386 bm1 | 387 bm2[2]*ones | 388 bm2[3]*ones
    wpack = nc.declare_dram_parameter("wpack", [H, WPK], fp32, isOutput=False)
    w1b = nc.declare_dram_parameter("w1b", [FD, H], fp32, isOutput=False)
    out = nc.declare_dram_parameter("out", [2, BS_LOCAL, OUT_W], fp32, isOutput=True)

    with tile.TileContext(nc) as tc:
        with (
            tc.tile_pool(name="sb", bufs=1) as pool,
            tc.tile_pool(name="ps", bufs=1, space="PSUM") as ppool,
        ):
            obs_t = pool.tile([BS_LOCAL, OBS_W], fp32)
            nc.sync.dma_start(obs_t[:], obs[:])
            wp = pool.tile([H, WPK], fp32)
            nc.sync.dma_start(wp[:], wpack[:])
            w1b_t = pool.tile([FD, H], fp32)
            nc.sync.dma_start(w1b_t[:], w1b[:])

            cm15 = pool.tile([BS_LOCAL, 1], fp32)
            nc.vector.memset(cm15[:], -1.5)
            # dummy transcendental: hoists ACT_TABLE_LOAD into the DMA wait
            warm = pool.tile([1, 1], fp32)
            nc.vector.memset(warm[:], 0.0)
            nc.scalar.activation(warm[:], warm[:], AF.Tanh)

            # Node pooling over the 14 used features: obs row is 32 node
            # blocks of 16; S[:, 2:16] = sum over nodes of cols 2:16.
            S = pool.tile([BS_LOCAL, 2 * FD], fp32)
            nc.vector.memset(S[:], 0.0)
            nc.vector.tensor_reduce(
                S[:, 2:FD],
                obs_t[:].rearrange("p (n c) -> p c n", c=FD)[:, 2:FD, :],
                axis=mybir.AxisListType.X,
                op=ALU.add,
            )
            # [128, 16] -> [16, 128] via DVE 32x32 block transposes (rows
            # 16:32 of T are transposed zero padding, never read).
            T = pool.tile([2 * FD, BS_LOCAL], fp32)
            for b in range(4):
                nc.vector.transpose(
                    T[:, 32 * b : 32 * (b + 1)], S[32 * b : 32 * (b + 1), :]
                )

            # Channel-major MLP chain: [ch, graphs] tiles, weights as lhsT,
            # relu+bias fused on DVE (out = max(psum + b, 0)).
            h1_ps = ppool.tile([H, BS_LOCAL], fp32)
            nc.tensor.matmul(h1_ps[:], w1b_t[:], T[0:FD, :], start=True, stop=True)
            h1 = pool.tile([H, BS_LOCAL], fp32)
            nc.vector.tensor_scalar(
                h1[:], h1_ps[:], wp[:, 384:385], 0.0, ALU.add, ALU.max
            )

            h2_ps = ppool.tile([H, BS_LOCAL], fp32)
            nc.tensor.matmul(h2_ps[:], wp[:, 0:H], h1[:], start=True, stop=True)
            h2 = pool.tile([H, BS_LOCAL], fp32)
            nc.vector.tensor_scalar(
                h2[:], h2_ps[:], wp[:, 385:386], 0.0, ALU.add, ALU.max
            )

            m_ps = ppool.tile([H, BS_LOCAL], fp32)
            nc.tensor.matmul(m_ps[:], wp[:, H : 2 * H], h2[:], start=True, stop=True)
            m = pool.tile([H, BS_LOCAL], fp32)
            nc.vector.tensor_scalar(
                m[:], m_ps[:], wp[:, 386:387], 0.0, ALU.add, ALU.max
            )

            # Final layer, node-replicated weights: lhsT = m [ch, graphs]
            # puts graphs on PSUM partitions; cols 0:64 = mu plane (bias
            # added host-side), 64:128 = log_std plane.
            o_ps = ppool.tile([BS_LOCAL, 2 * OUT_W], fp32)
            nc.tensor.matmul(o_ps[:], m[:], wp[:, 2 * H : 3 * H], start=True, stop=True)

            O = pool.tile([BS_LOCAL, 2 * OUT_W], fp32)
            nc.vector.tensor_copy(O[:, 0:OUT_W], o_ps[:, 0:OUT_W])
            nc.sync.dma_start(out[0], O[:, 0:OUT_W])

            # std = exp(3.5*tanh(ls + bm2_ls) - 1.5); bm2_ls alternates per
            # column (2n+c -> bm2[2+c]), applied as per-partition bias on
            # even/odd strided views.
            ls = o_ps[:, OUT_W : 2 * OUT_W].rearrange("p (n c) -> p n c", c=2)
            tls = pool.tile([BS_LOCAL, OUT_W], fp32)
            tlsv = tls[:].rearrange("p (n c) -> p n c", c=2)
            nc.scalar.activation(tlsv[:, :, 0], ls[:, :, 0], AF.Tanh, bias=wp[:, 387:388])
            nc.scalar.activation(tlsv[:, :, 1], ls[:, :, 1], AF.Tanh, bias=wp[:, 388:389])
            nc.scalar.activation(
                O[:, OUT_W : 2 * OUT_W], tls[:], AF.Exp, bias=cm15[:], scale=3.5
            )
            nc.sync.dma_start(out[1], O[:, OUT_W : 2 * OUT_W])

    nc.compile()
    return nc


def _get_nc():
    if "nc" not in _NC_CACHE:
        _NC_CACHE["nc"] = _build_bass()
    return _NC_CACHE["nc"]


def _prep_inputs(inputs):
    obs = np.ascontiguousarray(np.asarray(inputs["obs"], dtype=np.float32))
    W1 = np.asarray(inputs["W1"], dtype=np.float32)
    b1 = np.asarray(inputs["b1"], dtype=np.float32)
    W2 = np.asarray(inputs["W2"], dtype=np.float32)
    b2 = np.asarray(inputs["b2"], dtype=np.float32)
    Wm1 = np.asarray(inputs["Wm1"], dtype=np.float32)
    bm1 = np.asarray(inputs["bm1"], dtype=np.float32)
    Wm2 = np.asarray(inputs["Wm2"], dtype=np.float32)
    bm2 = np.asarray(inputs["bm2"], dtype=np.float32)

    d = np.float32(1.0) / np.float32(np.sqrt(np.float32(32.0)))
    norm2 = np.float32(d * d)              # GCN symmetric norm, all edges
    W1p = np.zeros((FD, H), np.float32)
    W1p[2:FD] = W1 * norm2                 # drops robot_loc cols 0:2, scales
    W2s = (W2 * np.float32(np.float32(32.0) * norm2)).astype(np.float32)
    Wm2r = np.concatenate([np.tile(Wm2[:, 0:2], NN), np.tile(Wm2[:, 2:4], NN)], axis=1)

    ones = np.ones((H, 1), np.float32)
    wpack = np.ascontiguousarray(
        np.concatenate(
            [
                W2s,
                Wm1,
                Wm2r,
                b1[:, None],
                b2[:, None],
                bm1[:, None],
                bm2[2] * ones,
                bm2[3] * ones,
            ],
            axis=1,
        ).astype(np.float32)
    )

    shared = {"wpack": wpack, "w1b": np.ascontiguousarray(W1p)}
    in_maps = []
    for c in range(NCORES):
        mm = dict(shared)
        mm["obs"] = obs[c * BS_LOCAL : (c + 1) * BS_LOCAL]
        in_maps.append(mm)
    return in_maps


def kernel(**inputs):
    from concourse.bass_utils import run_bass_kernel_spmd

    assert inputs["obs"].shape == (BS, OBS_W), inputs["obs"].shape
    nc = _get_nc()
    in_maps = _prep_inputs(inputs)
    res = run_bass_kernel_spmd(nc, in_maps, list(range(NCORES))).results
    out = np.empty((2, BS, OUT_W), np.float32)
    for c in range(NCORES):
        out[:, c * BS_LOCAL : (c + 1) * BS_LOCAL, :] = res[c]["out"]
    # mu-plane bias (bm2[0:2]) is applied here instead of on-device: it is
    # outside every nonlinearity so the host add is exact.
    bm2 = np.asarray(inputs["bm2"], dtype=np.float32)
    if bm2[0] != 0.0 or bm2[1] != 0.0:
        out[0] += np.tile(bm2[0:2], NN)[None, :]
    return out



# revision 6
# speedup vs baseline: 1.1547x; 1.1547x over previous
"""Trainium2 Bass kernel for nn_GCNNDiagGaussianActor.

Structural insight: the reference GNN runs GCNConv layers over a COMPLETE
graph of 32 nodes per sample with self-loops. Every node has degree exactly
32 and the symmetric GCN norm is 1/32 for every edge, so the gather +
segment_sum collapses to a per-graph mean broadcast to every node. Per graph:

    pooled = sum_n obs[g, n, 0:16]                   (cols 0:2 zeroed in W1)
    h1  = relu(pooled @ (W1/32) + b1)
    h2  = relu(h1 @ W2 + b2)
    m   = relu(h2 @ Wm1 + bm1)
    o   = m @ Wm2r + bm2r                            -> [128] per graph
    mu  = o[0:64];  std = exp(3.5 * tanh(o[64:128]) - 1.5)

Sharding: data-parallel over batch, 128 graphs per core = 128 SBUF
partitions; small weights replicated. The x32 node replication is folded
into the last GEMM by tiling Wm2's columns host-side.

v4 perf structure (vs v3, ~22.3us):
- whole datapath in bf16 (rel-err gate is 2e-2; bf16 lands ~1e-3): obs DMA
  halves to 128KB and every matmul runs 1 cycle/row instead of fp32's 4.
- 3 input DMAs issued on 3 different HWDGE engines (SP/ACT/DVE) so their
  ~630ns issue costs overlap instead of serializing on SP.
- node pooling as a 4-step binary tree of contiguous bf16 adds (DVE 4x
  mode) instead of one strided reduce; the 5th tree step is folded into
  the first matmul by duplicating W1 rows (K=32, same PE cost).
- bm2 applied on-device as a rank-1 PE accumulate (ones_row x bm2_row)
  into the output PSUM group, collapsing the even/odd split tanh pair of
  v3 into a single tanh over the whole log_std plane.
- PE p-state pre-warm: dummy matmuls on a junk tile keep the tensor
  engine busy during the obs DMA wait so real matmuls run at full clock.
- dummy tanh right after the DMA issues hoists the scalar engine's
  ACT_TABLE_LOAD (~1.3us) off the critical path.
- mu output DMA issues while the std tanh/exp still run.
"""

import numpy as np

NCORES = 8
BS = 1024
BS_LOCAL = BS // NCORES   # 128 graphs per core
NN = 32                   # nodes per graph
FD = 16                   # per-node obs width
OBS_W = NN * FD           # 512
H = 128                   # hidden width
OUT_W = 2 * NN            # 64 = ACT_DIM * NN
WPK = 3 * H               # wpack cols: W2 | Wm1 | Wm2r
WSW = 5 * H               # ws cols: w1b32 | bm2r | b1r | b2r | bm1r (rows on p0)
N_WARM_MM = 6             # PE p-state warm-up matmuls

_NC_CACHE = {}


def _build_bass():
    import concourse.bacc as bacc
    import concourse.mybir as mybir
    from concourse import tile

    fp32 = mybir.dt.float32
    bf16 = mybir.dt.bfloat16
    AF = mybir.ActivationFunctionType
    ALU = mybir.AluOpType

    nc = bacc.Bacc(None, target_bir_lowering=False)
    obs = nc.declare_dram_parameter("obs", [BS_LOCAL, OBS_W], bf16, isOutput=False)
    # wpk cols: 0:128 W2 | 128:256 Wm1 | 256:384 Wm2r
    wpk = nc.declare_dram_parameter("wpk", [H, WPK], bf16, isOutput=False)
    # ws: cols 0:128 = [W1/32; W1/32] (rows 0:2 and 16:18 zero); partition-0
    # rows: 128:256 bm2r | 256:384 b1 | 384:512 b2 | 512:640 bm1 (biases are
    # applied as rank-1 PE accumulates, keeping the DVE relus scalar-free).
    ws = nc.declare_dram_parameter("ws", [2 * FD, WSW], bf16, isOutput=False)
    out = nc.declare_dram_parameter("out", [2, BS_LOCAL, OUT_W], fp32, isOutput=True)

    with tile.TileContext(nc) as tc:
        with (
            tc.tile_pool(name="sb", bufs=1) as pool,
            tc.tile_pool(name="ps", bufs=1, space="PSUM") as ppool,
        ):
            obs_t = pool.tile([BS_LOCAL, OBS_W], bf16)
            wp = pool.tile([H, WPK], bf16)
            ws_t = pool.tile([2 * FD, WSW], bf16)
            # input DMAs on three different engines -> parallel issue
            # (SP/ACT are HWDGE; Pool is SWDGE but idle and off-path early)
            nc.sync.dma_start(obs_t[:], obs[:])
            nc.scalar.dma_start(wp[:], wpk[:])
            nc.gpsimd.dma_start(ws_t[:], ws[:])

            ones = pool.tile([1, H], bf16)
            nc.vector.memset(ones[:], 1.0)
            cm15 = pool.tile([BS_LOCAL, 1], fp32)
            nc.vector.memset(cm15[:], -1.5)
            warm = pool.tile([1, 1], fp32)
            nc.vector.memset(warm[:], 0.0)
            # dummy transcendental: hoists ACT_TABLE_LOAD into the DMA wait
            nc.scalar.activation(warm[:], warm[:], AF.Tanh)

            # PE p-state warm-up: junk matmuls during the obs DMA wait
            junk = pool.tile([BS_LOCAL, OBS_W], bf16)
            nc.gpsimd.memset(junk[:], 0.0)
            jp = ppool.tile([1, OBS_W], fp32)
            for _ in range(N_WARM_MM):
                nc.tensor.matmul(jp[:], junk[:, 0:1], junk[:], start=True, stop=True)

            # Node pooling: binary tree of contiguous bf16 adds (DVE 4x).
            # 512 -> 256 -> 128 -> 64 -> 32; last halving is folded into the
            # first matmul via duplicated W1 rows.
            A1 = pool.tile([BS_LOCAL, 256], bf16)
            nc.vector.tensor_add(A1[:], obs_t[:, 0:256], obs_t[:, 256:512])
            A2 = pool.tile([BS_LOCAL, 128], bf16)
            nc.vector.tensor_add(A2[:], A1[:, 0:128], A1[:, 128:256])
            A3 = pool.tile([BS_LOCAL, 64], bf16)
            nc.vector.tensor_add(A3[:], A2[:, 0:64], A2[:, 64:128])
            A4 = pool.tile([BS_LOCAL, 32], bf16)
            nc.vector.tensor_add(A4[:], A3[:, 0:32], A3[:, 32:64])

            # [128, 32] -> [32, 128] via 4 DVE 32x32 block transposes
            T = pool.tile([2 * FD, BS_LOCAL], bf16)
            for b in range(4):
                nc.vector.transpose(
                    T[:, 32 * b : 32 * (b + 1)], A4[32 * b : 32 * (b + 1), :]
                )

            # Channel-major MLP chain: [ch, graphs] tiles, weights as lhsT,
            # relu+bias fused on DVE (out = max(psum + b, 0)).
            h1_ps = ppool.tile([H, BS_LOCAL], fp32)
            nc.tensor.matmul(h1_ps[:], ws_t[:, 0:H], T[:], start=True, stop=False)
            nc.tensor.matmul(
                h1_ps[:], ws_t[0:1, 2 * H : 3 * H], ones[:], start=False, stop=True
            )
            h1 = pool.tile([H, BS_LOCAL], bf16)
            nc.vector.tensor_scalar_max(h1[:], h1_ps[:], 0.0)

            h2_ps = ppool.tile([H, BS_LOCAL], fp32)
            nc.tensor.matmul(h2_ps[:], wp[:, 0:H], h1[:], start=True, stop=False)
            nc.tensor.matmul(
                h2_ps[:], ws_t[0:1, 3 * H : 4 * H], ones[:], start=False, stop=True
            )
            h2 = pool.tile([H, BS_LOCAL], bf16)
            nc.vector.tensor_scalar_max(h2[:], h2_ps[:], 0.0)

            m_ps = ppool.tile([H, BS_LOCAL], fp32)
            nc.tensor.matmul(m_ps[:], wp[:, H : 2 * H], h2[:], start=True, stop=False)
            nc.tensor.matmul(
                m_ps[:], ws_t[0:1, 4 * H : 5 * H], ones[:], start=False, stop=True
            )
            m = pool.tile([H, BS_LOCAL], bf16)
            nc.vector.tensor_scalar_max(m[:], m_ps[:], 0.0)

            # Final layer: lhsT = m puts graphs on PSUM partitions; cols
            # 0:64 mu plane, 64:128 log_std plane. bm2 lands via a rank-1
            # accumulate (ones x bm2_row) into the same PSUM group.
            o_ps = ppool.tile([BS_LOCAL, 2 * OUT_W], fp32)
            nc.tensor.matmul(o_ps[:], m[:], wp[:, 2 * H : 3 * H], start=True, stop=False)
            nc.tensor.matmul(
                o_ps[:], ones[:], ws_t[0:1, H : 2 * H], start=False, stop=True
            )

            O = pool.tile([BS_LOCAL, 2 * OUT_W], fp32)
            nc.vector.tensor_copy(O[:, 0:OUT_W], o_ps[:, 0:OUT_W])
            nc.sync.dma_start(out[0], O[:, 0:OUT_W])

            # std = exp(3.5*tanh(ls) - 1.5), single tanh over the plane
            tls = pool.tile([BS_LOCAL, OUT_W], fp32)
            nc.scalar.activation(tls[:], o_ps[:, OUT_W : 2 * OUT_W], AF.Tanh)
            nc.scalar.activation(
                O[:, OUT_W : 2 * OUT_W], tls[:], AF.Exp, bias=cm15[:], scale=3.5
            )
            nc.sync.dma_start(out[1], O[:, OUT_W : 2 * OUT_W])

    nc.compile()
    return nc


def _get_nc():
    if "nc" not in _NC_CACHE:
        _NC_CACHE["nc"] = _build_bass()
    return _NC_CACHE["nc"]


def _prep_inputs(inputs):
    import ml_dtypes

    bf16 = ml_dtypes.bfloat16

    obs = np.asarray(inputs["obs"], dtype=np.float32)
    W1 = np.asarray(inputs["W1"], dtype=np.float32)
    b1 = np.asarray(inputs["b1"], dtype=np.float32)
    W2 = np.asarray(inputs["W2"], dtype=np.float32)
    b2 = np.asarray(inputs["b2"], dtype=np.float32)
    Wm1 = np.asarray(inputs["Wm1"], dtype=np.float32)
    bm1 = np.asarray(inputs["bm1"], dtype=np.float32)
    Wm2 = np.asarray(inputs["Wm2"], dtype=np.float32)
    bm2 = np.asarray(inputs["bm2"], dtype=np.float32)

    # GCN symmetric norm over the complete-graph-with-self-loops: 1/32 per
    # edge; folded into W1. Layer 2's mean over identical node features is
    # the identity, so W2 is used as-is.
    W1s = W1 * np.float32(1.0 / 32.0)
    w1b = np.zeros((FD, H), np.float32)
    w1b[2:FD] = W1s                        # drops robot_loc cols 0:2
    w1b32 = np.concatenate([w1b, w1b], axis=0)          # [32, 128], K-dup
    bm2r = np.concatenate([np.tile(bm2[0:2], NN), np.tile(bm2[2:4], NN)])

    ws = np.zeros((2 * FD, 5 * H), np.float32)
    ws[:, 0:H] = w1b32
    ws[0, H : 2 * H] = bm2r
    ws[0, 2 * H : 3 * H] = b1
    ws[0, 3 * H : 4 * H] = b2
    ws[0, 4 * H : 5 * H] = bm1
    ws = np.ascontiguousarray(ws.astype(bf16))

    Wm2r = np.concatenate([np.tile(Wm2[:, 0:2], NN), np.tile(Wm2[:, 2:4], NN)], axis=1)
    wpk = np.ascontiguousarray(
        np.concatenate([W2, Wm1, Wm2r], axis=1).astype(bf16)
    )

    obs16 = np.ascontiguousarray(obs.astype(bf16))
    shared = {"wpk": wpk, "ws": ws}
    in_maps = []
    for c in range(NCORES):
        mm = dict(shared)
        mm["obs"] = obs16[c * BS_LOCAL : (c + 1) * BS_LOCAL]
        in_maps.append(mm)
    return in_maps


def kernel(**inputs):
    from concourse.bass_utils import run_bass_kernel_spmd

    assert inputs["obs"].shape == (BS, OBS_W), inputs["obs"].shape
    nc = _get_nc()
    in_maps = _prep_inputs(inputs)
    res = run_bass_kernel_spmd(nc, in_maps, list(range(NCORES))).results
    out = np.empty((2, BS, OUT_W), np.float32)
    for c in range(NCORES):
        out[:, c * BS_LOCAL : (c + 1) * BS_LOCAL, :] = res[c]["out"]
    return out


# revision 8
# speedup vs baseline: 1.2245x; 1.0605x over previous
"""Trainium2 Bass kernel for nn_GCNNDiagGaussianActor.

Structural insight: the reference GNN runs GCNConv layers over a COMPLETE
graph of 32 nodes per sample with self-loops. Every node has degree exactly
32 and the symmetric GCN norm is 1/32 for every edge, so the gather +
segment_sum collapses to a per-graph mean broadcast to every node. Per graph:

    pooled = sum_n obs[g, n, 0:16]                   (cols 0:2 zeroed in W1)
    h1  = relu(pooled @ (W1/32) + b1)
    h2  = relu(h1 @ W2 + b2)
    m   = relu(h2 @ Wm1 + bm1)
    o   = m @ Wm2r + bm2r                            -> [128] per graph
    mu  = o[0:64];  std = exp(3.5 * tanh(o[64:128]) - 1.5)

Sharding: data-parallel over batch, 128 graphs per core = 128 SBUF
partitions; small weights replicated. The x32 node replication is folded
into the last GEMM by tiling Wm2's columns host-side.

v5 perf structure (v3 baseline ~22.3us, v4 ~19.3us):
- whole datapath in bf16 (rel-err gate is 2e-2; bf16 lands ~1e-3): obs DMA
  halves to 128KB and every matmul runs 1 cycle/row instead of fp32's 4.
- input DMAs split across SP and ACT HWDGE queues for parallel issue.
- node pooling: two contiguous bf16 adds (512->256->128) on DVE, then ONE
  PE transpose of the [128,128] partial-sum block (identity shipped in the
  weight pack), ACT copies PSUM->SBUF, and the remaining 8-way node sum is
  folded into the first matmul by host-tiling W1 rows to K=128. This
  replaces v4's 4-step tree + 4 stream transposes (~1.9us of DVE).
- b1/b2/bm1 fused into the DVE relus as fp32 bias columns (v4's rank-1 PE
  bias matmuls cost ~380ns each - output rows dominate, K is irrelevant).
- final GEMM split into log_std half (+ rank-1 bm2 accumulate) and mu
  half, so tanh starts ~400ns earlier and the mu copy/DMA overlaps it.
- PE p-state pre-warm: dummy matmuls on a DVE-memset junk tile keep the
  tensor engine busy through the obs DMA wait (full clock for real mms).
- dummy tanh right after the DMA issues hoists the scalar engine's
  ACT_TABLE_LOAD (~1.3us) off the critical path.
- mu output DMA (SP) issues while tanh/exp run; std DMA issues from ACT.
"""

import numpy as np

NCORES = 8
BS = 1024
BS_LOCAL = BS // NCORES   # 128 graphs per core
NN = 32                   # nodes per graph
FD = 16                   # per-node obs width
OBS_W = NN * FD           # 512
H = 128                   # hidden width
OUT_W = 2 * NN            # 64 = ACT_DIM * NN
# wpk cols: W2 | Wm1 | Wm2r | w1til8 | bm2ls row | identity
C_W2 = 0
C_WM1 = H
C_WM2 = 2 * H
C_W1 = 3 * H
C_BM2 = 4 * H            # row 0 only, 64 wide
C_ID = 4 * H + OUT_W
WPK = C_ID + H
N_WARM_MM = 6             # PE p-state warm-up matmuls

_NC_CACHE = {}


def _build_bass():
    import concourse.bacc as bacc
    import concourse.mybir as mybir
    from concourse import tile

    fp32 = mybir.dt.float32
    bf16 = mybir.dt.bfloat16
    AF = mybir.ActivationFunctionType
    ALU = mybir.AluOpType

    nc = bacc.Bacc(None, target_bir_lowering=False)
    obs = nc.declare_dram_parameter("obs", [BS_LOCAL, OBS_W], bf16, isOutput=False)
    wpk = nc.declare_dram_parameter("wpk", [H, WPK], bf16, isOutput=False)
    bcols = nc.declare_dram_parameter("bcols", [H, 4], fp32, isOutput=False)
    out = nc.declare_dram_parameter("out", [2, BS_LOCAL, OUT_W], fp32, isOutput=True)

    with tile.TileContext(nc) as tc:
        with (
            tc.tile_pool(name="sb", bufs=1) as pool,
            tc.tile_pool(name="ps", bufs=1, space="PSUM") as ppool,
        ):
            obs_t = pool.tile([BS_LOCAL, OBS_W], bf16)
            wp = pool.tile([H, WPK], bf16)
            bc = pool.tile([H, 4], fp32)
            # parallel issue: obs+bcols on SP, weights on ACT
            nc.sync.dma_start(obs_t[:], obs[:])
            nc.scalar.dma_start(wp[:], wpk[:])
            nc.sync.dma_start(bc[:], bcols[:])

            # junk tile for PE warm-up + small constants (DVE, off-path)
            junk = pool.tile([BS_LOCAL, OBS_W], bf16)
            nc.vector.memset(junk[:], 0.0)
            ones = pool.tile([1, H], bf16)
            nc.vector.memset(ones[:], 1.0)
            cm15 = pool.tile([BS_LOCAL, 1], fp32)
            nc.vector.memset(cm15[:], -1.5)
            warm = pool.tile([1, 1], fp32)
            nc.vector.memset(warm[:], 0.0)
            # dummy transcendental: hoists ACT_TABLE_LOAD into the DMA wait
            nc.scalar.activation(warm[:], warm[:], AF.Tanh)

            # PE p-state warm-up during the obs DMA wait
            jp = ppool.tile([1, OBS_W], fp32)
            for _ in range(N_WARM_MM):
                nc.tensor.matmul(jp[:], junk[:, 0:1], junk[:], start=True, stop=True)

            # Node pooling, stage 1: 512 -> 256 -> 128 contiguous bf16 adds.
            # A2[g, n1*16+d] = sum over nodes n = n1 (mod 8) of obs dim d.
            A1 = pool.tile([BS_LOCAL, 256], bf16)
            nc.vector.tensor_add(A1[:], obs_t[:, 0:256], obs_t[:, 256:512])
            A2 = pool.tile([BS_LOCAL, 128], bf16)
            nc.vector.tensor_add(A2[:], A1[:, 0:128], A1[:, 128:256])

            # Stage 2: one PE transpose (vs 4 DVE stream transposes), then
            # the 8-way node sum rides inside mm1 via host-tiled W1 (K=128).
            tp = ppool.tile([H, BS_LOCAL], bf16)
            nc.tensor.transpose(tp[:], A2[:], wp[:, C_ID : C_ID + H])
            B = pool.tile([H, BS_LOCAL], bf16)
            nc.scalar.copy(B[:], tp[:])

            # MLP chain: [ch, graphs] tiles, weights as lhsT, relu+bias
            # fused on DVE (out = max(psum + b, 0)).
            h1_ps = ppool.tile([H, BS_LOCAL], fp32)
            nc.tensor.matmul(h1_ps[:], wp[:, C_W1 : C_W1 + H], B[:], start=True, stop=True)
            h1 = pool.tile([H, BS_LOCAL], bf16)
            nc.vector.tensor_scalar(h1[:], h1_ps[:], bc[:, 0:1], 0.0, ALU.add, ALU.max)

            h2_ps = ppool.tile([H, BS_LOCAL], fp32)
            nc.tensor.matmul(h2_ps[:], wp[:, C_W2 : C_W2 + H], h1[:], start=True, stop=True)
            h2 = pool.tile([H, BS_LOCAL], bf16)
            nc.vector.tensor_scalar(h2[:], h2_ps[:], bc[:, 1:2], 0.0, ALU.add, ALU.max)

            m_ps = ppool.tile([H, BS_LOCAL], fp32)
            nc.tensor.matmul(m_ps[:], wp[:, C_WM1 : C_WM1 + H], h2[:], start=True, stop=True)
            m = pool.tile([H, BS_LOCAL], bf16)
            nc.vector.tensor_scalar(m[:], m_ps[:], bc[:, 2:3], 0.0, ALU.add, ALU.max)

            # Final layer, graphs on PSUM partitions. log_std half first
            # (+ rank-1 bm2 accumulate) so tanh starts earlier; mu half
            # second, its copy/DMA overlapping tanh/exp.
            o_ps = ppool.tile([BS_LOCAL, 2 * OUT_W], fp32)
            nc.tensor.matmul(
                o_ps[:, OUT_W : 2 * OUT_W],
                m[:],
                wp[:, C_WM2 + OUT_W : C_WM2 + 2 * OUT_W],
                start=True,
                stop=False,
            )
            nc.tensor.matmul(
                o_ps[:, OUT_W : 2 * OUT_W],
                ones[:],
                wp[0:1, C_BM2 : C_BM2 + OUT_W],
                start=False,
                stop=True,
            )
            nc.tensor.matmul(
                o_ps[:, 0:OUT_W],
                m[:],
                wp[:, C_WM2 : C_WM2 + OUT_W],
                start=True,
                stop=True,
            )

            # std = exp(3.5*tanh(ls) - 1.5); DMA from ACT right after exp
            O = pool.tile([BS_LOCAL, 2 * OUT_W], fp32)
            tls = pool.tile([BS_LOCAL, OUT_W], fp32)
            nc.scalar.activation(tls[:], o_ps[:, OUT_W : 2 * OUT_W], AF.Tanh)
            nc.vector.tensor_copy(O[:, 0:OUT_W], o_ps[:, 0:OUT_W])
            nc.sync.dma_start(out[0], O[:, 0:OUT_W])
            nc.scalar.activation(
                O[:, OUT_W : 2 * OUT_W], tls[:], AF.Exp, bias=cm15[:], scale=3.5
            )
            nc.scalar.dma_start(out[1], O[:, OUT_W : 2 * OUT_W])

    nc.compile()
    return nc


def _get_nc():
    if "nc" not in _NC_CACHE:
        _NC_CACHE["nc"] = _build_bass()
    return _NC_CACHE["nc"]


def _prep_inputs(inputs):
    import ml_dtypes

    bf16 = ml_dtypes.bfloat16

    obs = np.asarray(inputs["obs"], dtype=np.float32)
    W1 = np.asarray(inputs["W1"], dtype=np.float32)
    b1 = np.asarray(inputs["b1"], dtype=np.float32)
    W2 = np.asarray(inputs["W2"], dtype=np.float32)
    b2 = np.asarray(inputs["b2"], dtype=np.float32)
    Wm1 = np.asarray(inputs["Wm1"], dtype=np.float32)
    bm1 = np.asarray(inputs["bm1"], dtype=np.float32)
    Wm2 = np.asarray(inputs["Wm2"], dtype=np.float32)
    bm2 = np.asarray(inputs["bm2"], dtype=np.float32)

    # GCN symmetric norm over the complete-graph-with-self-loops: 1/32 per
    # edge; folded into W1. Layer 2's mean over identical node features is
    # the identity, so W2 is used as-is. w1til8 tiles W1 rows 8x so mm1
    # (K=128) also performs the final 8-way node sum of the pooling tree.
    w1b = np.zeros((FD, H), np.float32)
    w1b[2:FD] = W1 * np.float32(1.0 / 32.0)     # drops robot_loc cols 0:2
    w1til8 = np.tile(w1b, (8, 1))               # [128, 128]

    Wm2r = np.concatenate([np.tile(Wm2[:, 0:2], NN), np.tile(Wm2[:, 2:4], NN)], axis=1)
    bm2pad = np.zeros((H, OUT_W), np.float32)
    bm2pad[0] = np.tile(bm2[2:4], NN)           # log_std-plane bias row
    ident = np.eye(H, dtype=np.float32)

    wpk = np.ascontiguousarray(
        np.concatenate([W2, Wm1, Wm2r, w1til8, bm2pad, ident], axis=1).astype(bf16)
    )
    bcols = np.ascontiguousarray(
        np.stack([b1, b2, bm1, np.zeros(H, np.float32)], axis=1)
    )

    obs16 = np.ascontiguousarray(obs.astype(bf16))
    shared = {"wpk": wpk, "bcols": bcols}
    in_maps = []
    for c in range(NCORES):
        mm = dict(shared)
        mm["obs"] = obs16[c * BS_LOCAL : (c + 1) * BS_LOCAL]
        in_maps.append(mm)
    return in_maps


def kernel(**inputs):
    from concourse.bass_utils import run_bass_kernel_spmd

    assert inputs["obs"].shape == (BS, OBS_W), inputs["obs"].shape
    nc = _get_nc()
    in_maps = _prep_inputs(inputs)
    res = run_bass_kernel_spmd(nc, in_maps, list(range(NCORES))).results
    out = np.empty((2, BS, OUT_W), np.float32)
    for c in range(NCORES):
        out[:, c * BS_LOCAL : (c + 1) * BS_LOCAL, :] = res[c]["out"]
    # mu-plane bm2 is outside every nonlinearity -> exact host add
    bm2 = np.asarray(inputs["bm2"], dtype=np.float32)
    if bm2[0] != 0.0 or bm2[1] != 0.0:
        out[0] += np.tile(bm2[0:2], NN)[None, :]
    return out


# revision 9
# speedup vs baseline: 1.2599x; 1.0289x over previous
"""Trainium2 Bass kernel for nn_GCNNDiagGaussianActor.

Structural insight: the reference GNN runs GCNConv layers over a COMPLETE
graph of 32 nodes per sample with self-loops. Every node has degree exactly
32 and the symmetric GCN norm is 1/32 for every edge, so the gather +
segment_sum collapses to a per-graph mean broadcast to every node. Per graph:

    pooled = sum_n obs[g, n, 0:16]                   (cols 0:2 zeroed in W1)
    h1  = relu(pooled @ (W1/32) + b1)
    h2  = relu(h1 @ W2 + b2)
    m   = relu(h2 @ Wm1 + bm1)
    o   = m @ Wm2r + bm2r                            -> [128] per graph
    mu  = o[0:64];  std = exp(3.5 * tanh(o[64:128]) - 1.5)

Sharding: data-parallel over batch, 128 graphs per core = 128 SBUF
partitions; small weights replicated. The x32 node replication is folded
into the last GEMM by tiling Wm2's columns host-side.

v5 perf structure (v3 baseline ~22.3us, v4 ~19.3us):
- whole datapath in bf16 (rel-err gate is 2e-2; bf16 lands ~1e-3): obs DMA
  halves to 128KB and every matmul runs 1 cycle/row instead of fp32's 4.
- input DMAs split across SP and ACT HWDGE queues for parallel issue.
- node pooling: two contiguous bf16 adds (512->256->128) on DVE, then ONE
  PE transpose of the [128,128] partial-sum block (identity shipped in the
  weight pack), ACT copies PSUM->SBUF, and the remaining 8-way node sum is
  folded into the first matmul by host-tiling W1 rows to K=128. This
  replaces v4's 4-step tree + 4 stream transposes (~1.9us of DVE).
- b1/b2/bm1 fused into the DVE relus as fp32 bias columns (v4's rank-1 PE
  bias matmuls cost ~380ns each - output rows dominate, K is irrelevant).
- final GEMM split into log_std half (+ rank-1 bm2 accumulate) and mu
  half, so tanh starts ~400ns earlier and the mu copy/DMA overlaps it.
- PE p-state pre-warm: dummy matmuls on a DVE-memset junk tile keep the
  tensor engine busy through the obs DMA wait (full clock for real mms).
- dummy tanh right after the DMA issues hoists the scalar engine's
  ACT_TABLE_LOAD (~1.3us) off the critical path.
- mu output DMA (SP) issues while tanh/exp run; std DMA issues from ACT.
"""

import numpy as np

NCORES = 8
BS = 1024
BS_LOCAL = BS // NCORES   # 128 graphs per core
NN = 32                   # nodes per graph
FD = 16                   # per-node obs width
OBS_W = NN * FD           # 512
H = 128                   # hidden width
OUT_W = 2 * NN            # 64 = ACT_DIM * NN
# wpk cols: W2 | Wm1 | Wm2r | w1til8 | bm2ls row | identity
C_W2 = 0
C_WM1 = H
C_WM2 = 2 * H
C_W1 = 3 * H
C_BM2 = 4 * H            # row 0 only, 64 wide
C_ID = 4 * H + OUT_W
WPK = C_ID + H
N_WARM_MM = 5             # PE p-state warm-up matmuls

_NC_CACHE = {}


def _build_bass(with_bias):
    import concourse.bacc as bacc
    import concourse.mybir as mybir
    from concourse import tile

    fp32 = mybir.dt.float32
    bf16 = mybir.dt.bfloat16
    AF = mybir.ActivationFunctionType
    ALU = mybir.AluOpType

    nc = bacc.Bacc(None, target_bir_lowering=False)
    obs = nc.declare_dram_parameter("obs", [BS_LOCAL, OBS_W], bf16, isOutput=False)
    wpk = nc.declare_dram_parameter("wpk", [H, WPK], bf16, isOutput=False)
    if with_bias:
        bcols = nc.declare_dram_parameter("bcols", [H, 4], fp32, isOutput=False)
    out = nc.declare_dram_parameter("out", [2, BS_LOCAL, OUT_W], fp32, isOutput=True)

    with tile.TileContext(nc) as tc:
        with (
            tc.tile_pool(name="sb", bufs=1) as pool,
            tc.tile_pool(name="ps", bufs=1, space="PSUM") as ppool,
        ):
            obs_t = pool.tile([BS_LOCAL, OBS_W], bf16)
            wp = pool.tile([H, WPK], bf16)
            # parallel issue: obs (+bias cols) on SP, weights on ACT
            nc.sync.dma_start(obs_t[:], obs[:])
            nc.scalar.dma_start(wp[:], wpk[:])
            if with_bias:
                bc = pool.tile([H, 4], fp32)
                nc.sync.dma_start(bc[:], bcols[:])

            # junk tile for PE warm-up + small constants (DVE, off-path)
            junk = pool.tile([BS_LOCAL, OBS_W], bf16)
            nc.vector.memset(junk[:], 0.0)
            if with_bias:
                ones = pool.tile([1, H], bf16)
                nc.vector.memset(ones[:], 1.0)
            cm15 = pool.tile([BS_LOCAL, 1], fp32)
            nc.vector.memset(cm15[:], -1.5)
            warm = pool.tile([1, 1], fp32)
            nc.vector.memset(warm[:], 0.0)
            # dummy transcendental: hoists ACT_TABLE_LOAD into the DMA wait
            nc.scalar.activation(warm[:], warm[:], AF.Tanh)

            # PE p-state warm-up during the obs DMA wait
            jp = ppool.tile([1, OBS_W], fp32)
            for _ in range(N_WARM_MM):
                nc.tensor.matmul(jp[:], junk[:, 0:1], junk[:], start=True, stop=True)

            # Node pooling, stage 1: 512 -> 256 -> 128 contiguous bf16 adds.
            # A2[g, n1*16+d] = sum over nodes n = n1 (mod 8) of obs dim d.
            A1 = pool.tile([BS_LOCAL, 256], bf16)
            nc.vector.tensor_add(A1[:], obs_t[:, 0:256], obs_t[:, 256:512])
            A2 = pool.tile([BS_LOCAL, 128], bf16)
            nc.vector.tensor_add(A2[:], A1[:, 0:128], A1[:, 128:256])

            # Stage 2: one PE transpose (vs 4 DVE stream transposes), then
            # the 8-way node sum rides inside mm1 via host-tiled W1 (K=128).
            tp = ppool.tile([H, BS_LOCAL], bf16)
            nc.tensor.transpose(tp[:], A2[:], wp[:, C_ID : C_ID + H])
            B = pool.tile([H, BS_LOCAL], bf16)
            nc.vector.tensor_copy(B[:], tp[:])

            # MLP chain: [ch, graphs] tiles, weights as lhsT, relu+bias
            # fused on DVE (out = max(psum + b, 0)).
            def relu(dst, src, col):
                if with_bias:
                    nc.vector.tensor_scalar(
                        dst, src, bc[:, col : col + 1], 0.0, ALU.add, ALU.max
                    )
                else:
                    nc.vector.tensor_scalar_max(dst, src, 0.0)

            h1_ps = ppool.tile([H, BS_LOCAL], fp32)
            nc.tensor.matmul(h1_ps[:], wp[:, C_W1 : C_W1 + H], B[:], start=True, stop=True)
            h1 = pool.tile([H, BS_LOCAL], bf16)
            relu(h1[:], h1_ps[:], 0)

            h2_ps = ppool.tile([H, BS_LOCAL], fp32)
            nc.tensor.matmul(h2_ps[:], wp[:, C_W2 : C_W2 + H], h1[:], start=True, stop=True)
            h2 = pool.tile([H, BS_LOCAL], bf16)
            relu(h2[:], h2_ps[:], 1)

            m_ps = ppool.tile([H, BS_LOCAL], fp32)
            nc.tensor.matmul(m_ps[:], wp[:, C_WM1 : C_WM1 + H], h2[:], start=True, stop=True)
            m = pool.tile([H, BS_LOCAL], bf16)
            relu(m[:], m_ps[:], 2)

            # Final layer, graphs on PSUM partitions. log_std half first
            # (+ rank-1 bm2 accumulate) so tanh starts earlier; mu half
            # second, its copy/DMA overlapping tanh/exp.
            o_ps = ppool.tile([BS_LOCAL, 2 * OUT_W], fp32)
            nc.tensor.matmul(
                o_ps[:, OUT_W : 2 * OUT_W],
                m[:],
                wp[:, C_WM2 + OUT_W : C_WM2 + 2 * OUT_W],
                start=True,
                stop=not with_bias,
            )
            if with_bias:
                nc.tensor.matmul(
                    o_ps[:, OUT_W : 2 * OUT_W],
                    ones[:],
                    wp[0:1, C_BM2 : C_BM2 + OUT_W],
                    start=False,
                    stop=True,
                )
            nc.tensor.matmul(
                o_ps[:, 0:OUT_W],
                m[:],
                wp[:, C_WM2 : C_WM2 + OUT_W],
                start=True,
                stop=True,
            )

            # std = exp(3.5*tanh(ls) - 1.5); DMA from ACT right after exp
            O = pool.tile([BS_LOCAL, 2 * OUT_W], fp32)
            tls = pool.tile([BS_LOCAL, OUT_W], fp32)
            nc.scalar.activation(tls[:], o_ps[:, OUT_W : 2 * OUT_W], AF.Tanh)
            nc.vector.tensor_copy(O[:, 0:OUT_W], o_ps[:, 0:OUT_W])
            nc.sync.dma_start(out[0], O[:, 0:OUT_W])
            nc.scalar.activation(
                O[:, OUT_W : 2 * OUT_W], tls[:], AF.Exp, bias=cm15[:], scale=3.5
            )
            nc.scalar.dma_start(out[1], O[:, OUT_W : 2 * OUT_W])

    nc.compile()
    return nc


def _get_nc(with_bias):
    key = ("bias" if with_bias else "fast")
    if key not in _NC_CACHE:
        _NC_CACHE[key] = _build_bass(with_bias)
    return _NC_CACHE[key]


def _prep_inputs(inputs):
    import ml_dtypes

    bf16 = ml_dtypes.bfloat16

    obs = np.asarray(inputs["obs"], dtype=np.float32)
    W1 = np.asarray(inputs["W1"], dtype=np.float32)
    b1 = np.asarray(inputs["b1"], dtype=np.float32)
    W2 = np.asarray(inputs["W2"], dtype=np.float32)
    b2 = np.asarray(inputs["b2"], dtype=np.float32)
    Wm1 = np.asarray(inputs["Wm1"], dtype=np.float32)
    bm1 = np.asarray(inputs["bm1"], dtype=np.float32)
    Wm2 = np.asarray(inputs["Wm2"], dtype=np.float32)
    bm2 = np.asarray(inputs["bm2"], dtype=np.float32)

    # GCN symmetric norm over the complete-graph-with-self-loops: 1/32 per
    # edge; folded into W1. Layer 2's mean over identical node features is
    # the identity, so W2 is used as-is. w1til8 tiles W1 rows 8x so mm1
    # (K=128) also performs the final 8-way node sum of the pooling tree.
    w1b = np.zeros((FD, H), np.float32)
    w1b[2:FD] = W1 * np.float32(1.0 / 32.0)     # drops robot_loc cols 0:2
    w1til8 = np.tile(w1b, (8, 1))               # [128, 128]

    Wm2r = np.concatenate([np.tile(Wm2[:, 0:2], NN), np.tile(Wm2[:, 2:4], NN)], axis=1)
    bm2pad = np.zeros((H, OUT_W), np.float32)
    bm2pad[0] = np.tile(bm2[2:4], NN)           # log_std-plane bias row
    ident = np.eye(H, dtype=np.float32)

    wpk = np.ascontiguousarray(
        np.concatenate([W2, Wm1, Wm2r, w1til8, bm2pad, ident], axis=1).astype(bf16)
    )
    bcols = np.ascontiguousarray(
        np.stack([b1, b2, bm1, np.zeros(H, np.float32)], axis=1)
    )

    with_bias = bool(
        np.any(b1) or np.any(b2) or np.any(bm1) or np.any(bm2[2:4])
    )
    obs16 = np.ascontiguousarray(obs.astype(bf16))
    shared = {"wpk": wpk}
    if with_bias:
        shared["bcols"] = bcols
    in_maps = []
    for c in range(NCORES):
        mm = dict(shared)
        mm["obs"] = obs16[c * BS_LOCAL : (c + 1) * BS_LOCAL]
        in_maps.append(mm)
    return in_maps, with_bias


def kernel(**inputs):
    from concourse.bass_utils import run_bass_kernel_spmd

    assert inputs["obs"].shape == (BS, OBS_W), inputs["obs"].shape
    in_maps, with_bias = _prep_inputs(inputs)
    nc = _get_nc(with_bias)
    res = run_bass_kernel_spmd(nc, in_maps, list(range(NCORES))).results
    out = np.empty((2, BS, OUT_W), np.float32)
    for c in range(NCORES):
        out[:, c * BS_LOCAL : (c + 1) * BS_LOCAL, :] = res[c]["out"]
    # mu-plane bm2 is outside every nonlinearity -> exact host add
    bm2 = np.asarray(inputs["bm2"], dtype=np.float32)
    if bm2[0] != 0.0 or bm2[1] != 0.0:
        out[0] += np.tile(bm2[0:2], NN)[None, :]
    return out


# revision 10
# speedup vs baseline: 1.2826x; 1.0181x over previous
"""Trainium2 Bass kernel for nn_GCNNDiagGaussianActor.

Structural insight: the reference GNN runs GCNConv layers over a COMPLETE
graph of 32 nodes per sample with self-loops. Every node has degree exactly
32 and the symmetric GCN norm is 1/32 for every edge, so the gather +
segment_sum collapses to a per-graph mean broadcast to every node. Per graph:

    pooled = sum_n obs[g, n, 0:16]                   (cols 0:2 zeroed in W1)
    h1  = relu(pooled @ (W1/32) + b1)
    h2  = relu(h1 @ W2 + b2)
    m   = relu(h2 @ Wm1 + bm1)
    o   = m @ Wm2r + bm2r                            -> [128] per graph
    mu  = o[0:64];  std = exp(3.5 * tanh(o[64:128]) - 1.5)

Sharding: data-parallel over batch, 128 graphs per core = 128 SBUF
partitions; small weights replicated. The x32 node replication is folded
into the last GEMM by tiling Wm2's columns host-side.

v5 perf structure (v3 baseline ~22.3us, v4 ~19.3us):
- whole datapath in bf16 (rel-err gate is 2e-2; bf16 lands ~1e-3): obs DMA
  halves to 128KB and every matmul runs 1 cycle/row instead of fp32's 4.
- input DMAs split across SP and ACT HWDGE queues for parallel issue.
- node pooling: two contiguous bf16 adds (512->256->128) on DVE, then ONE
  PE transpose of the [128,128] partial-sum block (identity shipped in the
  weight pack), ACT copies PSUM->SBUF, and the remaining 8-way node sum is
  folded into the first matmul by host-tiling W1 rows to K=128. This
  replaces v4's 4-step tree + 4 stream transposes (~1.9us of DVE).
- b1/b2/bm1 fused into the DVE relus as fp32 bias columns (v4's rank-1 PE
  bias matmuls cost ~380ns each - output rows dominate, K is irrelevant).
- final GEMM split into log_std half (+ rank-1 bm2 accumulate) and mu
  half, so tanh starts ~400ns earlier and the mu copy/DMA overlaps it.
- PE p-state pre-warm: dummy matmuls on a DVE-memset junk tile keep the
  tensor engine busy through the obs DMA wait (full clock for real mms).
- dummy tanh right after the DMA issues hoists the scalar engine's
  ACT_TABLE_LOAD (~1.3us) off the critical path.
- mu output DMA (SP) issues while tanh/exp run; std DMA issues from ACT.
"""

import numpy as np

NCORES = 8
BS = 1024
BS_LOCAL = BS // NCORES   # 128 graphs per core
NN = 32                   # nodes per graph
FD = 16                   # per-node obs width
OBS_W = NN * FD           # 512
H = 128                   # hidden width
OUT_W = 2 * NN            # 64 = ACT_DIM * NN
# wpk cols: W2 | Wm1 | Wm2r | w1til8 | bm2ls row | identity
C_W2 = 0
C_WM1 = H
C_WM2 = 2 * H
C_W1 = 3 * H
C_BM2 = 4 * H            # row 0 only, 64 wide
C_ID = 4 * H + OUT_W
WPK = C_ID + H
N_WARM_MM = 5             # PE p-state warm-up matmuls

_NC_CACHE = {}


def _build_bass(with_bias):
    import concourse.bacc as bacc
    import concourse.mybir as mybir
    from concourse import tile

    fp32 = mybir.dt.float32
    bf16 = mybir.dt.bfloat16
    AF = mybir.ActivationFunctionType
    ALU = mybir.AluOpType

    nc = bacc.Bacc(None, target_bir_lowering=False)
    obs = nc.declare_dram_parameter("obs", [BS_LOCAL, OBS_W], bf16, isOutput=False)
    wpk = nc.declare_dram_parameter("wpk", [H, WPK], bf16, isOutput=False)
    if with_bias:
        bcols = nc.declare_dram_parameter("bcols", [H, 4], fp32, isOutput=False)
    out = nc.declare_dram_parameter("out", [2, BS_LOCAL, OUT_W], fp32, isOutput=True)

    with tile.TileContext(nc) as tc:
        with (
            tc.tile_pool(name="sb", bufs=1) as pool,
            tc.tile_pool(name="ps", bufs=1, space="PSUM") as ppool,
        ):
            obs_t = pool.tile([BS_LOCAL, OBS_W], bf16)
            wp = pool.tile([H, WPK], bf16)
            # parallel issue: obs (+bias cols) on SP, weights on ACT
            nc.sync.dma_start(obs_t[:], obs[:])
            nc.scalar.dma_start(wp[:], wpk[:])
            if with_bias:
                bc = pool.tile([H, 4], fp32)
                nc.sync.dma_start(bc[:], bcols[:])

            # junk tile for PE warm-up + small constants (DVE, off-path)
            junk = pool.tile([BS_LOCAL, OBS_W], bf16)
            nc.vector.memset(junk[:], 0.0)
            if with_bias:
                ones = pool.tile([1, H], bf16)
                nc.vector.memset(ones[:], 1.0)
            cm15 = pool.tile([BS_LOCAL, 1], fp32)
            nc.vector.memset(cm15[:], -1.5)
            warm = pool.tile([1, 1], fp32)
            nc.vector.memset(warm[:], 0.0)
            # dummy transcendental: hoists ACT_TABLE_LOAD into the DMA wait
            nc.scalar.activation(warm[:], warm[:], AF.Tanh)

            # PE p-state warm-up during the obs DMA wait; the last short
            # matmul keeps PE busy up to the A2 transpose arrival
            jp = ppool.tile([1, OBS_W], fp32)
            for _ in range(N_WARM_MM):
                nc.tensor.matmul(jp[:], junk[:, 0:1], junk[:], start=True, stop=True)
            nc.tensor.matmul(jp[:, 0:128], junk[:, 0:1], junk[:, 0:128], start=True, stop=True)

            # Node pooling, stage 1: 512 -> 256 -> 128 contiguous bf16 adds.
            # A2[g, n1*16+d] = sum over nodes n = n1 (mod 8) of obs dim d.
            A1 = pool.tile([BS_LOCAL, 256], bf16)
            nc.vector.tensor_add(A1[:], obs_t[:, 0:256], obs_t[:, 256:512])
            A2 = pool.tile([BS_LOCAL, 128], bf16)
            nc.vector.tensor_add(A2[:], A1[:, 0:128], A1[:, 128:256])

            # Stage 2: one PE transpose (vs 4 DVE stream transposes), then
            # the 8-way node sum rides inside mm1 via host-tiled W1 (K=128).
            tp = ppool.tile([H, BS_LOCAL], bf16)
            nc.tensor.transpose(tp[:], A2[:], wp[:, C_ID : C_ID + H])
            B = pool.tile([H, BS_LOCAL], bf16)
            nc.vector.tensor_copy(B[:], tp[:])

            # MLP chain: [ch, graphs] tiles, weights as lhsT, relu+bias
            # fused on DVE (out = max(psum + b, 0)).
            def relu(dst, src, col):
                if with_bias:
                    nc.vector.tensor_scalar(
                        dst, src, bc[:, col : col + 1], 0.0, ALU.add, ALU.max
                    )
                else:
                    nc.vector.tensor_scalar_max(dst, src, 0.0)

            h1_ps = ppool.tile([H, BS_LOCAL], fp32)
            nc.tensor.matmul(h1_ps[:], wp[:, C_W1 : C_W1 + H], B[:], start=True, stop=True)
            h1 = pool.tile([H, BS_LOCAL], bf16)
            relu(h1[:], h1_ps[:], 0)

            h2_ps = ppool.tile([H, BS_LOCAL], fp32)
            nc.tensor.matmul(h2_ps[:], wp[:, C_W2 : C_W2 + H], h1[:], start=True, stop=True)
            h2 = pool.tile([H, BS_LOCAL], bf16)
            relu(h2[:], h2_ps[:], 1)

            m_ps = ppool.tile([H, BS_LOCAL], fp32)
            nc.tensor.matmul(m_ps[:], wp[:, C_WM1 : C_WM1 + H], h2[:], start=True, stop=True)
            m = pool.tile([H, BS_LOCAL], bf16)
            relu(m[:], m_ps[:], 2)

            # Final layer, graphs on PSUM partitions. log_std half first
            # (+ rank-1 bm2 accumulate) so tanh starts earlier; mu half
            # second, its copy/DMA overlapping tanh/exp.
            # separate PSUM tiles for the two halves: a shared tile would
            # put a false reader-reader dep between tanh and the mu copy
            ls_ps = ppool.tile([BS_LOCAL, OUT_W], fp32)
            mu_ps = ppool.tile([BS_LOCAL, OUT_W], fp32)
            nc.tensor.matmul(
                ls_ps[:],
                m[:],
                wp[:, C_WM2 + OUT_W : C_WM2 + 2 * OUT_W],
                start=True,
                stop=not with_bias,
            )
            if with_bias:
                nc.tensor.matmul(
                    ls_ps[:],
                    ones[:],
                    wp[0:1, C_BM2 : C_BM2 + OUT_W],
                    start=False,
                    stop=True,
                )
            nc.tensor.matmul(
                mu_ps[:],
                m[:],
                wp[:, C_WM2 : C_WM2 + OUT_W],
                start=True,
                stop=True,
            )

            # std = exp(3.5*tanh(ls) - 1.5); DMA from ACT right after exp
            O = pool.tile([BS_LOCAL, 2 * OUT_W], fp32)
            tls = pool.tile([BS_LOCAL, OUT_W], fp32)
            nc.scalar.activation(tls[:], ls_ps[:], AF.Tanh)
            nc.vector.tensor_copy(O[:, 0:OUT_W], mu_ps[:])
            nc.sync.dma_start(out[0], O[:, 0:OUT_W])
            nc.scalar.activation(
                O[:, OUT_W : 2 * OUT_W], tls[:], AF.Exp, bias=cm15[:], scale=3.5
            )
            nc.scalar.dma_start(out[1], O[:, OUT_W : 2 * OUT_W])

    nc.compile()
    return nc


def _get_nc(with_bias):
    key = ("bias" if with_bias else "fast")
    if key not in _NC_CACHE:
        _NC_CACHE[key] = _build_bass(with_bias)
    return _NC_CACHE[key]


def _prep_inputs(inputs):
    import ml_dtypes

    bf16 = ml_dtypes.bfloat16

    obs = np.asarray(inputs["obs"], dtype=np.float32)
    W1 = np.asarray(inputs["W1"], dtype=np.float32)
    b1 = np.asarray(inputs["b1"], dtype=np.float32)
    W2 = np.asarray(inputs["W2"], dtype=np.float32)
    b2 = np.asarray(inputs["b2"], dtype=np.float32)
    Wm1 = np.asarray(inputs["Wm1"], dtype=np.float32)
    bm1 = np.asarray(inputs["bm1"], dtype=np.float32)
    Wm2 = np.asarray(inputs["Wm2"], dtype=np.float32)
    bm2 = np.asarray(inputs["bm2"], dtype=np.float32)

    # GCN symmetric norm over the complete-graph-with-self-loops: 1/32 per
    # edge; folded into W1. Layer 2's mean over identical node features is
    # the identity, so W2 is used as-is. w1til8 tiles W1 rows 8x so mm1
    # (K=128) also performs the final 8-way node sum of the pooling tree.
    w1b = np.zeros((FD, H), np.float32)
    w1b[2:FD] = W1 * np.float32(1.0 / 32.0)     # drops robot_loc cols 0:2
    w1til8 = np.tile(w1b, (8, 1))               # [128, 128]

    Wm2r = np.concatenate([np.tile(Wm2[:, 0:2], NN), np.tile(Wm2[:, 2:4], NN)], axis=1)
    bm2pad = np.zeros((H, OUT_W), np.float32)
    bm2pad[0] = np.tile(bm2[2:4], NN)           # log_std-plane bias row
    ident = np.eye(H, dtype=np.float32)

    wpk = np.ascontiguousarray(
        np.concatenate([W2, Wm1, Wm2r, w1til8, bm2pad, ident], axis=1).astype(bf16)
    )
    bcols = np.ascontiguousarray(
        np.stack([b1, b2, bm1, np.zeros(H, np.float32)], axis=1)
    )

    with_bias = bool(
        np.any(b1) or np.any(b2) or np.any(bm1) or np.any(bm2[2:4])
    )
    obs16 = np.ascontiguousarray(obs.astype(bf16))
    shared = {"wpk": wpk}
    if with_bias:
        shared["bcols"] = bcols
    in_maps = []
    for c in range(NCORES):
        mm = dict(shared)
        mm["obs"] = obs16[c * BS_LOCAL : (c + 1) * BS_LOCAL]
        in_maps.append(mm)
    return in_maps, with_bias


def kernel(**inputs):
    from concourse.bass_utils import run_bass_kernel_spmd

    assert inputs["obs"].shape == (BS, OBS_W), inputs["obs"].shape
    in_maps, with_bias = _prep_inputs(inputs)
    nc = _get_nc(with_bias)
    res = run_bass_kernel_spmd(nc, in_maps, list(range(NCORES))).results
    out = np.empty((2, BS, OUT_W), np.float32)
    for c in range(NCORES):
        out[:, c * BS_LOCAL : (c + 1) * BS_LOCAL, :] = res[c]["out"]
    # mu-plane bm2 is outside every nonlinearity -> exact host add
    bm2 = np.asarray(inputs["bm2"], dtype=np.float32)
    if bm2[0] != 0.0 or bm2[1] != 0.0:
        out[0] += np.tile(bm2[0:2], NN)[None, :]
    return out
